# revision 46
# baseline (speedup 1.0000x reference)
"""Trainium2 Bass kernel for nn_DegModel (dense_cnn, per-pixel 21x21 kernel predictor).

Math (see reference):
  trunk of 1x1 convs on z -> logits (441 taps per pixel) -> softmax -> per-pixel
  21x21 kernel applied to reflect-padded x at stride 4.
Returns (out, kernel): out (B,3,64,64) f32, kernel (B,441,64,64) f32.

Sharding: pure data parallel over batch B=16 -> 2 images per NeuronCore x 8 cores.

Device layout choices:
  * Channels (64) live on partitions for the trunk; the two spatial halves of a
    batch are stacked to fill 128 partitions, with block-diagonalized weights
    (lhsT = blockdiag(W.T, W.T)) so trunk matmuls use the full 128x128 array.
    Matmul operands are float32r (full PE rate, ~1e-3 rel precision).
  * Logits are computed pixel-major: lhsT = h-slice (64ch x 128px), rhs = w_out.T
    (64 x 442, zero-padded for fp32r evenness) -> PSUM tile (128 px x 442 taps);
    softmax reduces along the free dim (ACT exp with fused accum, DVE
    reciprocal + tensor_scalar normalize).
  * The patch tensor ("XS") is staged host-side (bf16) so each partition
    (= output pixel slot) holds its own strided rows of the padded image; the
    21x21 patch for pixel-chunk k is a contiguous 441-element slice - no
    on-chip gather. Partitions 64..127 (odd output rows) are the same data
    shifted by 4 image rows, built with one SBUF->SBUF DMA.
  * Apply = bf16 tensor_mul (DVE 2x mode, 3 channels batched) + add-reduce.
    The reduce is split between DVE tensor_reduce and ACT activation-accum
    to balance engine load. The bf16 copy of the normalized kernel runs on
    GPSIMD. Kernel output (kout) stays fp32-exact.
"""

import os

import numpy as np
import ml_dtypes

import concourse.bacc as bacc
import concourse.bass as bass
import concourse.tile as tile
from concourse import mybir
from concourse.bass_utils import run_bass_kernel_spmd

F32 = mybir.dt.float32
BF16 = mybir.dt.bfloat16
ds = bass.ds

B, C, H, W = 16, 3, 256, 256
KS, SC, PAD = 21, 4, 10
K2 = KS * KS  # 441
K2P = K2 + 1  # 442 (even, for fp32r matmul + 2x DVE modes)
NF = 64
NB = 8
HL, WL = H // SC, W // SC  # 64, 64
NPX = HL * WL  # 4096
NCORES = 8
BLOC = B // NCORES  # 2
NROW = 273  # staged padded-image rows (max row index used is 272)
XSF = NROW * KS  # 5733 free elems per XS partition
XSF_E = XSF + 1  # 5734, even so per-channel byte offsets stay 4B-aligned
XSF1 = 168 * 31 + K2  # 5649: extent of the shifted (odd-row) copy
NCHUNK = NPX // 128  # 32 pixel chunks of 128 per batch

# Matmul operand dtype: float32r streams fp32 operands at full PE rate with
# ~tf32 multiply precision. (Plain float32 = exact but 4 cycles/row.)
MM_DT = mybir.dt.float32r

# Every Nth chunk's apply-reduce runs on ScalarE instead of VectorE (balance).
ACT_REDUCE_EVERY = int(os.environ.get("DEG_ACT_REDUCE_EVERY", "3"))
# Residual adds via PSUM accumulation (frees DVE) instead of DVE tensor_add.
PSUM_RESIDUAL = bool(int(os.environ.get("DEG_PSUM_RESIDUAL", "0")))
POOL_MULT = int(os.environ.get("DEG_POOL_MULT", "0"))  # 0=off, N=every Nth chunk

_CACHE: dict = {}


def _emit_body(ctx, tc, z2, xs, wt, wo, kout, oout):
    nc = tc.nc
    Act = mybir.ActivationFunctionType

    consts = ctx.enter_context(tc.tile_pool(name="consts", bufs=1))
    zpool = ctx.enter_context(tc.tile_pool(name="zp", bufs=2))
    hpool = ctx.enter_context(tc.tile_pool(name="hp", bufs=3 + 2 * bool(int(os.environ.get("DEG_DUAL_TRUNK", "0")))))
    rpool = ctx.enter_context(tc.tile_pool(name="rp", bufs=3))
    xspool = ctx.enter_context(tc.tile_pool(name="xsp", bufs=2))
    kerpool = ctx.enter_context(tc.tile_pool(name="kerp", bufs=4))
    epool = ctx.enter_context(tc.tile_pool(name="ep", bufs=4))
    kfpool = ctx.enter_context(tc.tile_pool(name="kfp", bufs=4))
    spool = ctx.enter_context(tc.tile_pool(name="sp", bufs=8))
    ppool = ctx.enter_context(tc.tile_pool(name="pp", bufs=4))
    opool = ctx.enter_context(tc.tile_pool(name="op", bufs=1))
    if PSUM_RESIDUAL:
        phacc = ctx.enter_context(tc.tile_pool(name="phacc", bufs=2, space="PSUM"))
        ptrunk = ctx.enter_context(tc.tile_pool(name="ptrunk", bufs=1, space="PSUM"))
    else:
        ptrunk = ctx.enter_context(tc.tile_pool(name="ptrunk", bufs=6, space="PSUM"))
    plog = ctx.enter_context(tc.tile_pool(name="plog", bufs=2, space="PSUM"))

    wt_sb = consts.tile([128, 17, 128], MM_DT)
    nc.sync.dma_start(out=wt_sb[:, 0, :], in_=wt.rearrange("l k m -> k l m")[:, 0, :])
    nc.sync.dma_start(out=wt_sb[:, 1:17, :], in_=wt.rearrange("l k m -> k l m")[:, 1:17, :])
    # w_out.T replicated into both partition halves so the rhs base partition
    # can match either h-slice (matmul requires equal base partitions).
    wo_sb = consts.tile([128, K2P], MM_DT)
    nc.sync.dma_start(out=wo_sb[0:64, :], in_=wo)
    nc.sync.dma_start(out=wo_sb[64:128, :], in_=wo)

    def mm(out_ap, lhsT, rhs):
        nc.tensor.matmul(out_ap, lhsT=lhsT, rhs=rhs, start=True, stop=True)

    # --- per-batch input loads (both batches up front; pools hold 2) ---
    z_sbs, xs_sbs = [], []
    for b in range(BLOC):
        z_sb = zpool.tile([128, 2048], MM_DT, tag="z")
        nc.sync.dma_start(out=z_sb, in_=z2[b])
        xs_sb = xspool.tile([128, C, XSF_E], BF16, tag="xs")
        for c in range(C):
            nc.sync.dma_start(out=xs_sb[0:64, c, 0:XSF], in_=xs[b, c])
            nc.sync.dma_start(out=xs_sb[64:128, c, 0:XSF1], in_=xs_sb[0:64, c, 84 : 84 + XSF1])
        z_sbs.append(z_sb)
        xs_sbs.append(xs_sb)

    def make_trunk_units(b):
        """Emission units (closures) for batch b's trunk; returns (units, state).
        state["h"] holds the latest h tile; final after all units ran."""
        state = {}

        def u_start():
            h_new = hpool.tile([128, 2048], MM_DT, tag="h")
            state["h"] = h_new

        def u_win(s):
            def f():
                sw = 2048 // (4 if (b == 0 and HYBRID_TRUNK) else 2)
                hp = ptrunk.tile([128, sw], F32, tag="tp")
                for j in range(sw // 512):
                    mm(hp[:, ds(512 * j, 512)], wt_sb[:, 0, :], z_sbs[b][:, ds(sw * s + 512 * j, 512)])
                nc.scalar.activation(state["h"][:, ds(sw * s, sw)], hp, Act.Copy)
            return f

        def u_newh():
            state["hprev"] = state["h"]
            h_new = hpool.tile([128, 2048], MM_DT, tag="h")
            state["h"] = h_new

        def u_block(i, s):
            def f():
                sw = 2048 // (4 if (b == 0 and HYBRID_TRUNK) else 2)
                h, h2 = state["hprev"], state["h"]
                r1p = ptrunk.tile([128, sw], F32, tag="tp")
                for j in range(sw // 512):
                    mm(r1p[:, ds(512 * j, 512)], wt_sb[:, 1 + 2 * i, :], h[:, ds(sw * s + 512 * j, 512)])
                rs = rpool.tile([128, sw], MM_DT, tag="rs")
                nc.scalar.activation(rs, r1p, Act.Relu)
                r2p = ptrunk.tile([128, sw], F32, tag="tp")
                for j in range(sw // 512):
                    mm(r2p[:, ds(512 * j, 512)], wt_sb[:, 2 + 2 * i, :], rs[:, ds(512 * j, 512)])
                nc.vector.tensor_add(h2[:, ds(sw * s, sw)], h[:, ds(sw * s, sw)], r2p)
            return f

        nstrip = 4 if (b == 0 and HYBRID_TRUNK) else 2
        units = [u_start] + [u_win(s) for s in range(nstrip)]
        for i in range(NB):
            units.append(u_newh)
            for s in range(nstrip):
                units.append(u_block(i, s))
        return units, state

    def emit_chunk(b, k, h, xs_sb, osb, kview):
        if k < 16:
            hsl = h[0:64, ds(128 * k, 128)]
            wosl = wo_sb[0:64, :]
        else:
            hsl = h[64:128, ds(128 * (k - 16), 128)]
            wosl = wo_sb[64:128, :]
        lp = plog.tile([128, K2P], F32, tag="lp")
        mm(lp, hsl, wosl)
        e = epool.tile([128, K2], F32, tag="e")
        ssum = spool.tile([128, 1], F32, tag="ssum")
        nc.scalar.activation(e, lp[:, 0:K2], Act.Exp, accum_out=ssum)
        rec = spool.tile([128, 1], F32, tag="rec")
        nc.vector.reciprocal(rec, ssum)
        kf = kfpool.tile([128, K2P], F32, tag="kf")  # fp32 normalized kernel
        nc.vector.tensor_scalar_mul(kf[:, 0:K2], e, rec)
        pool_mult = POOL_MULT_NUM and (k * POOL_MULT_NUM) % POOL_MULT_DEN < POOL_MULT_NUM
        # bf16 copy for the apply (GPSIMD, otherwise idle)
        kb = kerpool.tile([128, K2P], BF16, tag="kb")
        nc.gpsimd.tensor_copy(kb[:, 0:K2], kf[:, 0:K2])
        nc.sync.dma_start(out=kview[:, k, :], in_=kf[:, 0:K2])

        # apply: prod = ker * patch (bf16, 3 channels), then add-reduce
        prod = ppool.tile([128, C, K2], BF16, tag="prod")
        mul_eng = nc.gpsimd if pool_mult else nc.vector
        mul_eng.tensor_mul(
            prod,
            kb[:, 0:K2].unsqueeze(1).broadcast_to([128, C, K2]),
            xs_sb[:, :, ds(168 * k, K2)],
        )
        if (k * ACT_REDUCE_NUM) % ACT_REDUCE_DEN < ACT_REDUCE_NUM:
            pdump = ppool.tile([128, C, K2], BF16, tag="pdump")
            for c in range(C):
                nc.scalar.activation(
                    pdump[:, c, :],
                    prod[:, c, :],
                    Act.Copy,
                    accum_out=osb[:, 3 * k + c : 3 * k + c + 1],
                )
        else:
            nc.vector.tensor_reduce(
                osb[:, ds(3 * k, C)],
                prod,
                axis=mybir.AxisListType.X,
                op=mybir.AluOpType.add,
            )

    # --- both trunks interleaved (independent chains), or b0-then-weave ---
    units0, st0 = make_trunk_units(0)
    units1, st1 = make_trunk_units(1)
    ui = 0
    if DUAL_TRUNK:
        for i, u in enumerate(units0):
            u()
            units1[i]()
        ui = len(units1)
    else:
        for u in units0:
            u()

    osb0 = opool.tile([128, NCHUNK * C], F32, tag="osb0")
    osb1 = opool.tile([128, NCHUNK * C], F32, tag="osb1")
    osbs = [osb0, osb1]
    kviews = [kout[b].rearrange("(k p) t -> p k t", p=128) for b in range(BLOC)]

    for k in range(NCHUNK):
        emit_chunk(0, k, st0["h"], xs_sbs[0], osbs[0], kviews[0])
        if k >= INTERLEAVE_START:
            while ui < len(units1) and ui <= (k - INTERLEAVE_START + 1) * len(units1) // max(1, NCHUNK - INTERLEAVE_START):
                units1[ui]()
                ui += 1
    while ui < len(units1):
        units1[ui]()
        ui += 1
    nc.sync.dma_start(
        out=oout[0].rearrange("(k p) c -> p k c", p=128),
        in_=osbs[0].rearrange("p (k c) -> p k c", c=C),
    )
    for k in range(NCHUNK):
        emit_chunk(1, k, st1["h"], xs_sbs[1], osbs[1], kviews[1])
    nc.sync.dma_start(
        out=oout[1].rearrange("(k p) c -> p k c", p=128),
        in_=osbs[1].rearrange("p (k c) -> p k c", c=C),
    )


def build_program():
    from contextlib import ExitStack

    nc = bacc.Bacc("TRN2", target_bir_lowering=False, debug=False)
    z2 = nc.dram_tensor("z2", [BLOC, 128, 2048], MM_DT, kind="ExternalInput").ap()
    xs = nc.dram_tensor("xs", [BLOC, C, 64, XSF], BF16, kind="ExternalInput").ap()
    wt = nc.dram_tensor("wt", [17, 128, 128], MM_DT, kind="ExternalInput").ap()
    wo = nc.dram_tensor("wo", [64, K2P], MM_DT, kind="ExternalInput").ap()
    kout = nc.dram_tensor("kout", [BLOC, NPX, K2], F32, kind="ExternalOutput").ap()
    oout = nc.dram_tensor("oout", [BLOC, NPX, C], F32, kind="ExternalOutput").ap()
    with tile.TileContext(nc) as tc:
        with ExitStack() as ctx:
            _emit_body(ctx, tc, z2, xs, wt, wo, kout, oout)
    nc.compile()
    return nc


def get_program():
    if "nc" not in _CACHE:
        _CACHE["nc"] = build_program()
    return _CACHE["nc"]


def host_prep(x, z, w_in, res_w1, res_w2, w_out):
    """Build the per-core device input maps."""
    x = np.ascontiguousarray(np.asarray(x, np.float32))
    z = np.asarray(z, np.float32)

    # reflect-pad and stage patches: xs[b,c,xl, row*21+j] = xp[b,c,row, 4*xl+j]
    xp = np.pad(x, ((0, 0), (0, 0), (PAD, PAD), (PAD, PAD)), mode="reflect")
    st = xp.strides
    xsv = np.lib.stride_tricks.as_strided(
        xp,
        shape=(B, C, WL, NROW, KS),
        strides=(st[0], st[1], st[3] * SC, st[2], st[3]),
    )
    xs = np.ascontiguousarray(xsv).reshape(B, C, WL, XSF).astype(ml_dtypes.bfloat16)

    # z in stacked block layout: (B, 128, 2048)
    zr = z.reshape(B, NF, NPX)
    z2 = np.concatenate([zr[:, :, :2048], zr[:, :, 2048:]], axis=1)
    z2 = np.ascontiguousarray(z2)

    # block-diagonal transposed trunk weights (lhsT for out = W @ h)
    wt = np.zeros((17, 128, 128), np.float32)

    def bd(wmat):
        out = np.zeros((128, 128), np.float32)
        out[:NF, :NF] = wmat.T
        out[NF:, NF:] = wmat.T
        return out

    wt[0] = bd(w_in)
    for i in range(NB):
        wt[1 + 2 * i] = bd(res_w1[i])
        wt[2 + 2 * i] = bd(res_w2[i])
    wo = np.zeros((NF, K2P), np.float32)  # padded to 442 for fp32r evenness
    wo[:, :K2] = np.asarray(w_out, np.float32).T

    in_maps = []
    for core in range(NCORES):
        sl = slice(BLOC * core, BLOC * (core + 1))
        in_maps.append(
            {
                "z2": np.ascontiguousarray(z2[sl]),
                "xs": np.ascontiguousarray(xs[sl]),
                "wt": wt,
                "wo": wo,
            }
        )
    return in_maps


def kernel(x, z, w_in, b_in, res_w1, res_b1, res_w2, res_b2, w_out, b_out):
    # All biases are zeros by problem construction; the device program omits them.
    for name, bias in (("b_in", b_in), ("res_b1", res_b1), ("res_b2", res_b2), ("b_out", b_out)):
        if np.abs(np.asarray(bias)).max() != 0:
            raise NotImplementedError(f"nonzero bias {name} not supported")

    in_maps = host_prep(x, z, w_in, res_w1, res_w2, w_out)
    nc = get_program()
    trace = bool(int(os.environ.get("DEG_TRACE", "0")))
    if trace:
        try:  # the axon NTFF profile hook is absent in some containers
            from antenv.axon_hooks import get_axon_ntff_profile_hook  # noqa: F401
        except ImportError:
            trace = False
    res = run_bass_kernel_spmd(nc, in_maps, list(range(NCORES)), trace=trace)
    _CACHE["last_result"] = res

    kfull = np.concatenate([res.results[i]["kout"] for i in range(NCORES)], axis=0)
    ofull = np.concatenate([res.results[i]["oout"] for i in range(NCORES)], axis=0)
    kernel_out = np.ascontiguousarray(
        kfull.reshape(B, HL, WL, K2).transpose(0, 3, 1, 2)
    )
    out = np.ascontiguousarray(ofull.reshape(B, HL, WL, C).transpose(0, 3, 1, 2))
    return out, kernel_out
